# revision 1
# baseline (speedup 1.0000x reference)
"""CrossScaleGNN forward on 8 Trainium2 NeuronCores (pure data parallel).

Reference computation (B=32768, S=6, D=512, fp32):
    adj = softmax(scale_emb @ scale_emb.T)            # [6, 6]
    msg = einsum('ij,bjd->bid', adj, h)               # [B, 6, D]
    m   = gelu(msg @ W1.T + b1) @ W2.T + b2           # exact (erf) gelu
    out = layer_norm(h + m) * gamma + beta            # gamma=1, beta=0

Device strategy (per core, batch shard of 4096 rows = 24576 tokens):
  - h is shipped as fp16; tokens are processed in chunks of whole
    batch-groups (21 or 16 groups of 6 tokens) so the scale-mixing can ride
    a single PE matmul: the stationary operand is a chunk of h
    (tokens x d-slice), the moving operand is the block-diagonal matrix
    kron(I, adj.T), and the output is msg^T (d x tokens) - the mixing and
    the transpose into matmul layout are fused into one PE pass.
  - the 2-layer MLP runs in fp16 on the PE with fp32 PSUM accumulation
    (N=504-wide moving operands, one PSUM bank per tile).
  - m^T is transposed back to token-major with fp16 identity matmuls; the
    residual add (x = m + h) rides the single DVE pass that stages the
    PSUM result to SBUF, which also releases the PSUM bank early.
  - LayerNorm stats come from bn_stats/bn_aggr; the scalar tail
    (sqrt -> reciprocal -> scale/bias) of each macro-tile is emitted after
    the NEXT macro-tile's head so the in-order ACT queue never blocks the
    critical msg^T copies; the normalization itself is one fused DVE
    tensor_scalar (x*rs + (-mu*rs)).

adj (a 6x6 softmax of parameter products, O(S^2 D) work) is computed on the
host in float64; everything O(B) runs on device. Emission is software-
pipelined: head (loads, mix-transpose, copies) -> previous tile's LN tail
-> MLP/transpose/stats, over a single 8-bank PSUM ring.
"""

import numpy as np

B, S, D = 32768, 6, 512
N_CORES = 8
B_PER_CORE = B // N_CORES           # 4096 batch rows
TOK_PER_CORE = B_PER_CORE * S       # 24576 tokens
# chunk schedule: 192 chunks of 21 batch-groups + 4 chunks of 16 -> 4096
CHUNK_GROUPS = [21] * 192 + [16] * 4
assert sum(CHUNK_GROUPS) == B_PER_CORE
MTILE = 4                            # chunks per macro-tile (one PSUM generation)
assert len(CHUNK_GROUPS) % MTILE == 0

_CACHE = {}


def _split_waits(nc, max_waits=1):
    """Split excess sync-waits onto preceding NoOps (walrus in this build
    rejects instructions carrying more than one sync-wait command)."""
    import concourse.mybir as mybir

    n = 0
    for f in nc.m.functions:
        for blk in f.blocks:
            insts = blk.instructions
            idx = 0
            while idx < len(insts):
                inst = insts[idx]
                si = inst.sync_info
                if si is not None and si.on_wait is not None and len(si.on_wait) > max_waits:
                    waits = list(si.on_wait)
                    extra, keep = waits[:-max_waits], waits[-max_waits:]
                    k = 0
                    while extra:
                        chunk, extra = extra[:max_waits], extra[max_waits:]
                        nop = mybir.InstNoOp(
                            name=f"{inst.name}-wsplit{k}",
                            sync_info=mybir.SyncInfo(on_wait=chunk, on_update=[]),
                            bass_nofuse=True,
                            engine=inst.engine,
                        )
                        insts.insert(idx, nop)
                        idx += 1
                        k += 1
                    inst.sync_info = mybir.SyncInfo(
                        on_wait=keep, on_update=list(si.on_update or [])
                    )
                    n += 1
                idx += 1
    return n


def _build_program():
    import concourse.bass as bass
    import concourse.mybir as mybir
    import concourse.tile as tile

    F32, BF16 = mybir.dt.float32, mybir.dt.float16
    AF = mybir.ActivationFunctionType

    nc = bass.Bass("TRN2", target_bir_lowering=False, debug=False,
                   num_devices=N_CORES)

    h_d = nc.declare_dram_parameter("h", [TOK_PER_CORE, D], BF16, isOutput=False)
    out_d = nc.declare_dram_parameter("out", [TOK_PER_CORE, D], F32, isOutput=True)
    bd_d = nc.declare_dram_parameter("BD", [126, 126], BF16, isOutput=False)
    i126_d = nc.declare_dram_parameter("I126", [126, 126], BF16, isOutput=False)
    i128_d = nc.declare_dram_parameter("I128", [128, 128], BF16, isOutput=False)
    w1t_d = nc.declare_dram_parameter("W1T", [4, 128, D], BF16, isOutput=False)
    w2t_d = nc.declare_dram_parameter("W2T", [4, 128, D], BF16, isOutput=False)
    b1_d = nc.declare_dram_parameter("B1", [128, 4], F32, isOutput=False)
    b2_d = nc.declare_dram_parameter("B2", [128, 4], F32, isOutput=False)

    with tile.TileContext(nc) as tc:
        with (
            tc.tile_pool(name="const", bufs=1) as cp,
            tc.tile_pool(name="work", bufs=8) as wp,
            tc.tile_pool(name="ps", bufs=8, space="PSUM") as ps,
        ):
            bd = cp.tile([126, 126], BF16, tag="bd")
            nc.sync.dma_start(bd[:], bd_d[:])
            i126 = cp.tile([126, 126], BF16, tag="i126")
            nc.sync.dma_start(i126[:], i126_d[:])
            i128 = cp.tile([128, 128], BF16, tag="i128")
            nc.sync.dma_start(i128[:], i128_d[:])
            b1t = cp.tile([128, 4], F32, tag="b1t")
            nc.sync.dma_start(b1t[:], b1_d[:])
            b2t = cp.tile([128, 4], F32, tag="b2t")
            nc.sync.dma_start(b2t[:], b2_d[:])
            b1c = [b1t[:, k:k + 1] for k in range(4)]
            b2c = [b2t[:, k:k + 1] for k in range(4)]
            w1t, w2t = [], []

            def load_weights():
                for k in range(4):
                    w = cp.tile([128, D], BF16, tag=f"w1t{k}")
                    nc.sync.dma_start(w[:], w1t_d[k])
                    w1t.append(w)
                    w = cp.tile([128, D], BF16, tag=f"w2t{k}")
                    nc.sync.dma_start(w[:], w2t_d[k])
                    w2t.append(w)

            n_mt = len(CHUNK_GROUPS) // MTILE

            def emit_head(mt):
                """load, cast, fused mix+transpose, PSUM->SBUF copies."""
                tok0 = sum(6 * g for g in CHUNK_GROUPS[:mt * MTILE])
                tcs = [6 * g for g in CHUNK_GROUPS[mt * MTILE:(mt + 1) * MTILE]]
                S_tok = sum(tcs)
                offs = [sum(tcs[:c]) for c in range(MTILE)]

                h_bf = []
                for c, Tc in enumerate(tcs):
                    hb = wp.tile([Tc, D], BF16, tag="h")
                    nc.sync.dma_start(hb[:], h_d[tok0 + offs[c]: tok0 + offs[c] + Tc])
                    h_bf.append(hb)
                h_nat = h_bf

                msgT_ps = []
                for k in range(4):
                    p = ps.tile([128, S_tok], F32, tag="ps")
                    for c, Tc in enumerate(tcs):
                        nc.tensor.matmul(
                            p[:, offs[c]:offs[c] + Tc],
                            h_bf[c][:, k * 128:(k + 1) * 128],
                            bd[:Tc, :Tc],
                            start=True, stop=True,
                        )
                    msgT_ps.append(p)
                msgT_bf = []
                for k in range(4):
                    t = wp.tile([128, S_tok], BF16, tag="msgT")
                    nc.scalar.copy(t[:], msgT_ps[k][:])
                    msgT_bf.append(t)
                return (tok0, tcs, S_tok, offs, h_nat, h_bf, msgT_bf)

            def emit_rest(state):
                tok0, tcs, S_tok, offs, h_nat, h_bf, msgT_bf = state
                # 4. layer 1 + gelu, wavefront order over (m, k): early
                # matmuls only need the first msgT copies, and each z1[m]
                # still finishes in m order so the gelus pipeline
                wave = sorted(((m, k) for m in range(4) for k in range(4)),
                              key=lambda mk: (mk[0] + mk[1], mk[0]))
                a1_bf = []
                z1_ps = []
                for m in range(4):
                    p = ps.tile([128, S_tok], F32, tag="ps")
                    z1_ps.append(p)
                for m, k in wave:
                    nc.tensor.matmul(z1_ps[m][:], w1t[k][:, m * 128:(m + 1) * 128],
                                     msgT_bf[k][:], start=(k == 0), stop=(k == 3))
                for m in range(4):
                    a = wp.tile([128, S_tok], BF16, tag="a1")
                    nc.scalar.activation(a[:], z1_ps[m][:], AF.Gelu, bias=b1c[m], scale=1.0)
                    a1_bf.append(a)

                # 5. layer 2, wavefront over (k, o)
                xT = []
                xps = []
                for k in range(4):
                    p = ps.tile([128, S_tok], F32, tag="ps")
                    xps.append(p)
                for k, o in wave:
                    nc.tensor.matmul(xps[k][:], w2t[o][:, k * 128:(k + 1) * 128],
                                     a1_bf[o][:], start=(o == 0), stop=(o == 3))
                for k in range(4):
                    # m^T (+b2) -> SBUF fp16
                    t = wp.tile([128, S_tok], BF16, tag="xT")
                    nc.scalar.activation(t[:], xps[k][:], AF.Identity, bias=b2c[k], scale=1.0)
                    xT.append(t)

                # 6. transpose back to token-major, stage x = m + h to SBUF
                # (the PSUM bank is released by this single DVE pass, which
                # also performs the fp32 residual add) and take LN stats
                tail = []
                pnat = []
                for Tc in tcs:
                    pn = ps.tile([Tc, D], F32, tag="ps")
                    pnat.append(pn)
                for k in range(4):
                    for c, Tc in enumerate(tcs):
                        nc.tensor.matmul(
                            pnat[c][:, k * 128:(k + 1) * 128],
                            xT[k][:, offs[c]:offs[c] + Tc],
                            i128[:],
                            start=True, stop=True,
                        )
                for c, Tc in enumerate(tcs):
                    p = pnat[c]
                    x_sb = wp.tile([Tc, D], F32, tag="xsb")
                    nc.vector.tensor_tensor(x_sb[:], p[:], h_nat[c][:],
                                            mybir.AluOpType.add)
                    st6 = wp.tile([Tc, 6], F32, tag="st6")
                    nc.vector.bn_stats(st6[:], x_sb[:])
                    st2 = wp.tile([Tc, 2], F32, tag="st2")
                    nc.vector.bn_aggr(st2[:], st6[:])
                    tail.append((c, Tc, x_sb, st2))
                return tok0, offs, tail

            def emit_tail(tail_state):
                """LN scalar chain + normalize + store. Emitted after the
                next mtile's head so the ACT-queue sqrt (which waits on the
                DVE stats chain) cannot block the next msgT copies."""
                tok0, offs, tail = tail_state
                for c, Tc, x_sb, st2 in tail:
                    veps = wp.tile([Tc, 1], F32, tag="veps")
                    nc.vector.tensor_scalar_add(veps[:], st2[:, 1:2], 1e-5)
                    sd = wp.tile([Tc, 1], F32, tag="sd")
                    nc.scalar.activation(sd[:], veps[:], AF.Sqrt)
                    rs = wp.tile([Tc, 1], F32, tag="rs")
                    nc.vector.reciprocal(rs[:], sd[:])
                    negmurs = wp.tile([Tc, 1], F32, tag="negmurs")
                    nc.vector.tensor_scalar(negmurs[:], st2[:, 0:1], rs[:], -1.0,
                                            mybir.AluOpType.mult, mybir.AluOpType.mult)
                    o = wp.tile([Tc, D], F32, tag="out")
                    nc.vector.tensor_scalar(o[:], x_sb[:], rs[:], negmurs[:],
                                            mybir.AluOpType.mult, mybir.AluOpType.add)
                    nc.sync.dma_start(out_d[tok0 + offs[c]: tok0 + offs[c] + Tc], o[:])

            state0 = emit_head(0)
            load_weights()
            tail_state = emit_rest(state0)
            for mt in range(1, n_mt):
                state = emit_head(mt)
                emit_tail(tail_state)
                tail_state = emit_rest(state)
            emit_tail(tail_state)

    _split_waits(nc)
    return nc


def _host_params(scale_emb, W1, b1, W2, b2):
    se = scale_emb.astype(np.float64)
    logits = se @ se.T
    logits -= logits.max(-1, keepdims=True)
    e = np.exp(logits)
    adj = (e / e.sum(-1, keepdims=True)).astype(np.float32)   # [6, 6]
    BDm = np.kron(np.eye(21, dtype=np.float32), adj.T).astype(np.float16)
    W1T = np.ascontiguousarray(W1.T).astype(np.float16).reshape(4, 128, D)
    W2T = np.ascontiguousarray(W2.T).astype(np.float16).reshape(4, 128, D)
    return {
        "BD": BDm,
        "I126": np.eye(126, dtype=np.float32).astype(np.float16),
        "I128": np.eye(128, dtype=np.float32).astype(np.float16),
        "W1T": W1T,
        "W2T": W2T,
        "B1": np.ascontiguousarray(b1.astype(np.float32).reshape(4, 128).T),
        "B2": np.ascontiguousarray(b2.astype(np.float32).reshape(4, 128).T),
    }


def _run(nc, in_maps, trace=False):
    from concourse.bass_utils import run_bass_kernel_spmd

    if trace:
        try:
            return run_bass_kernel_spmd(nc, in_maps,
                                        core_ids=list(range(N_CORES)),
                                        trace=True)
        except (ImportError, ModuleNotFoundError):
            pass  # no NTFF hook on this axon client; run untraced
    return run_bass_kernel_spmd(nc, in_maps, core_ids=list(range(N_CORES)))


def kernel(h, scale_emb, W1, b1, W2, b2, gamma, beta, _trace=False):
    h = np.asarray(h, dtype=np.float32)
    assert h.shape == (B, S, D)

    if "nc" not in _CACHE:
        _CACHE["nc"] = _build_program()
    nc = _CACHE["nc"]

    params = _host_params(np.asarray(scale_emb), np.asarray(W1), np.asarray(b1),
                          np.asarray(W2), np.asarray(b2))
    h2 = np.ascontiguousarray(h.reshape(B * S, D), dtype=np.float16)
    in_maps = []
    for i in range(N_CORES):
        m = dict(params)
        m["h"] = h2[i * TOK_PER_CORE:(i + 1) * TOK_PER_CORE]
        in_maps.append(m)

    res = _run(nc, in_maps, trace=_trace)
    out = np.empty((B * S, D), dtype=np.float32)
    for i in range(N_CORES):
        out[i * TOK_PER_CORE:(i + 1) * TOK_PER_CORE] = res.results[i]["out"]
    out = out.reshape(B, S, D)

    gamma = np.asarray(gamma, dtype=np.float32)
    beta = np.asarray(beta, dtype=np.float32)
    if not (np.all(gamma == 1.0) and np.all(beta == 0.0)):
        out = out * gamma + beta
    if _trace:
        _CACHE["last_result"] = res
    return out

